# revision 3
# baseline (speedup 1.0000x reference)
"""Trainium2 Bass kernel for 16-head MultiHeadAttention (B=2, S=2048, D=1024, f32).

Sharding: 8 cores = 2 (batch) x 4 (head groups of 4 heads).
Each core gets a col-shard of Wq/Wk/Wv ([1024,256]) + row-shard of Wo ([256,1024]),
computes a full [2048,1024] partial output; the host sums 4 partials per batch.

On-device pipeline (transposed layouts, seq on the free axis):
  QT/KT = Wpair^T @ x^T            -> [128(=2 heads x 64), 2048] f32r
  VT    = Wv_pair^T @ xv^T, then PE-transposed to V_aug [j, head, 65]
          (65th column = ones so AV emits softmax denominators for free)
  sT    = KT_h^T-slice @ QT_h-slice     (K=64 matmuls, PSUM [j=128, q=512])
  expT  = exp(0.125 * sT) via ACT straight from PSUM [128,1024] spans -> f32r
  outT/rowsum = V_aug^T @ expT          (M=65: rows 0-63 outT, row 64 rowsum)
  recip: rowsum rows -> DMA partition-gather -> lane-parallel reciprocal
         -> K=1 ones-matmul broadcast -> multiplied into outT (f32r)
  partial = outT^T @ Wo_shard           (K=128 over stacked head pairs)

All matmuls run as float32r (TF32-like 11-bit mantissa): full PE speed at
near-fp32 accuracy. Host pre-rounds inputs to the fp32r grid.
"""

import sys

import numpy as np

if "/opt/trn_rl_repo" not in sys.path:
    sys.path.insert(0, "/opt/trn_rl_repo")

import concourse.bacc as bacc
import concourse.mybir as mybir
import concourse.tile as tile
from concourse.masks import make_identity

F32 = mybir.dt.float32
F32R = mybir.dt.float32r

B, S, D, H = 2, 2048, 1024, 16
DK = D // H          # 64
HL = 4               # heads per core
DG = HL * DK         # 256
SCALE = 0.125        # 1/sqrt(DK)

ET = D // 128        # 8 e-tiles
JT = S // 128        # 16 j-tiles
QC = S // 512        # 4 q-chunks
JG = 2               # j-tiles per exp group
NG = JT // JG        # exp groups per (h, c)


def _round_fp32r(x: np.ndarray) -> np.ndarray:
    """Round f32 to the fp32r grid (11-bit mantissa, RNE), like walrus fp32_to_fp32r."""
    u = x.view(np.uint32).astype(np.uint64)
    u = (u + 0x7FF + ((u >> 12) & 1)) & np.uint64(0xFFFFF000)
    return u.astype(np.uint32).view(np.float32)


def _build_nc():
    nc = bacc.Bacc("TRN2", target_bir_lowering=False, debug=False)

    xq = nc.dram_tensor("xq", [D, S], F32, kind="ExternalInput").ap()
    xk = nc.dram_tensor("xk", [D, S], F32, kind="ExternalInput").ap()
    xv = nc.dram_tensor("xv", [D, S], F32, kind="ExternalInput").ap()
    wq = nc.dram_tensor("wq", [D, DG], F32, kind="ExternalInput").ap()
    wk = nc.dram_tensor("wk", [D, DG], F32, kind="ExternalInput").ap()
    wv = nc.dram_tensor("wv", [D, DG], F32, kind="ExternalInput").ap()
    wo = nc.dram_tensor("wo", [DG, D], F32, kind="ExternalInput").ap()
    out = nc.dram_tensor("out", [S, D], F32, kind="ExternalOutput").ap()

    with tile.TileContext(nc) as tc:
        with (
            tc.tile_pool(name="wpool", bufs=1) as wpool,
            tc.tile_pool(name="xin", bufs=2) as xin,
            tc.tile_pool(name="proj", bufs=1) as proj,
            tc.tile_pool(name="expp", bufs=6) as expp,
            tc.tile_pool(name="stp", bufs=6) as stp,
            tc.tile_pool(name="work", bufs=2) as work,
            tc.tile_pool(name="small", bufs=1) as small,
        ):
            # ---- constants / resident weights ------------------------------
            wq_sb = [wpool.tile([128, DG], F32R, tag=f"wq{e}", name=f"wq{e}")
                     for e in range(ET)]
            wk_sb = [wpool.tile([128, DG], F32R, tag=f"wk{e}", name=f"wk{e}")
                     for e in range(ET)]
            wv_sb = [wpool.tile([128, DG], F32R, tag=f"wv{e}", name=f"wv{e}")
                     for e in range(ET)]
            for e in range(ET):
                sl = slice(e * 128, (e + 1) * 128)
                nc.sync.dma_start(wq_sb[e], wq.bitcast(F32R)[sl, :])
                nc.sync.dma_start(wk_sb[e], wk.bitcast(F32R)[sl, :])
                nc.sync.dma_start(wv_sb[e], wv.bitcast(F32R)[sl, :])
            wo_sb = [wpool.tile([128, D], F32R, tag=f"wo{p}", name=f"wo{p}")
                     for p in range(2)]
            for p in range(2):
                nc.sync.dma_start(wo_sb[p], wo.bitcast(F32R)[p * 128:(p + 1) * 128, :])

            ones16 = wpool.tile([16, 64], F32, tag="ones16", name="ones16")
            nc.vector.memset(ones16, 1.0)
            ones_r = wpool.tile([16, 64], F32R, tag="ones_r", name="ones_r")
            nc.vector.tensor_copy(ones_r, ones16)
            ones_col = wpool.tile([128, 64], F32, tag="ones_col", name="ones_col")
            nc.vector.memset(ones_col, 1.0)
            ident_f = wpool.tile([128, 128], F32, tag="ident_f", name="ident_f")
            make_identity(nc, ident_f)
            ident = wpool.tile([128, 128], F32R, tag="ident", name="ident")
            nc.vector.tensor_copy(ident, ident_f)

            # ---- persistent activation tiles -------------------------------
            kt_sb = [proj.tile([128, S], F32R, tag=f"kt{p}", name=f"kt{p}")
                     for p in range(2)]
            qt_sb = [proj.tile([128, S], F32R, tag=f"qt{p}", name=f"qt{p}")
                     for p in range(2)]
            # V_aug: [j=128, j2, head, 65], two tiles cover jt = 0..7 / 8..15
            v_sb = [proj.tile([128, JT // 2, HL, DK + 1], F32R,
                              tag=f"v{hh}", name=f"v{hh}") for hh in range(2)]

            # ---- phase A: projections (8 PSUM accumulators, x streamed) ----
            vt_sb = []
            with tc.tile_pool(name="ps_a", bufs=1, space="PSUM") as ps_a:
                def proj_pairs(x_dram, w_tiles, dst_tiles, nm):
                    accs = [ps_a.tile([128, 512], F32, tag="pa", bufs=8,
                                      name=f"acc_{nm}_{p}_{c}")
                            for p in range(2) for c in range(QC)]
                    for e in range(ET):
                        xt = xin.tile([128, S], F32R, tag="xs", name=f"x_{nm}{e}")
                        nc.sync.dma_start(
                            xt, x_dram.bitcast(F32R)[e * 128:(e + 1) * 128, :])
                        for p in range(2):
                            for c in range(QC):
                                nc.tensor.matmul(
                                    accs[p * QC + c],
                                    w_tiles[e][:, p * 128:(p + 1) * 128],
                                    xt[:, c * 512:(c + 1) * 512],
                                    start=(e == 0), stop=(e == ET - 1),
                                )
                    for p in range(2):
                        for c in range(QC):
                            nc.vector.tensor_copy(
                                dst_tiles[p][:, c * 512:(c + 1) * 512],
                                accs[p * QC + c],
                            )

                proj_pairs(xk, wk_sb, kt_sb, "k")
                proj_pairs(xq, wq_sb, qt_sb, "q")
                # VT (same transposed layout), PE-transposed to V_aug below
                vt_sb = [proj.tile([128, S], F32R, tag=f"ot{p}", name=f"vt{p}")
                         for p in range(2)]
                proj_pairs(xv, wv_sb, vt_sb, "v")

            with tc.tile_pool(name="ps_t", bufs=2, space="PSUM") as ps_t:
                for p in range(2):
                    for jt in range(JT):
                        pt = ps_t.tile([128, 128], F32R, tag="pt", name=f"pt{p}_{jt}")
                        nc.tensor.transpose(
                            pt, vt_sb[p][:, jt * 128:(jt + 1) * 128], ident)
                        hh, j2 = divmod(jt, JT // 2)
                        nc.vector.tensor_copy(
                            v_sb[hh][:, j2, 2 * p:2 * p + 2, 0:DK],
                            pt.rearrange("j (h d) -> j h d", h=2),
                        )
                for hh in range(2):
                    nc.vector.tensor_copy(
                        v_sb[hh][:, :, :, DK:DK + 1],
                        ones_col[:, 0:32].rearrange("p (a b) -> p a b", a=8)[:, :, :, None],
                    )

            # ---- phases B+C: attention + output projection ------------------
            outt_sb = [proj.tile([128, S], F32R, tag=f"ot{p}", name=f"outt{p}")
                       for p in range(2)]

            with (
                tc.tile_pool(name="ps_sc", bufs=2, space="PSUM") as ps_sc,
                tc.tile_pool(name="ps_av", bufs=2, space="PSUM") as ps_av,
                tc.tile_pool(name="ps_r", bufs=1, space="PSUM") as ps_r,
                tc.tile_pool(name="ps_wo", bufs=1, space="PSUM") as ps_wo,
            ):
                for h in range(HL):
                    p, i = divmod(h, 2)
                    hsl = slice(i * 64, (i + 1) * 64)
                    av_stages = []
                    for c in range(QC):
                        csl = slice(c * 512, (c + 1) * 512)
                        exp_tiles = []
                        for g in range(NG):
                            sc = ps_sc.tile([128, JG * 512], F32, tag="sc",
                                            name=f"sc{h}_{c}_{g}")
                            for jj in range(JG):
                                jt = g * JG + jj
                                nc.tensor.matmul(
                                    sc[:, jj * 512:(jj + 1) * 512],
                                    kt_sb[p][hsl, jt * 128:(jt + 1) * 128],
                                    qt_sb[p][hsl, csl],
                                    start=True, stop=True,
                                )
                            ex = expp.tile([128, JG, 512], F32R, tag="ex",
                                           name=f"ex{h}_{c}_{g}")
                            nc.scalar.activation(
                                out=ex,
                                in_=sc.rearrange("p (j q) -> p j q", j=JG),
                                func=mybir.ActivationFunctionType.Exp,
                                scale=SCALE,
                            )
                            exp_tiles.append(ex)
                        av = ps_av.tile([128, 512], F32, tag="av", name=f"av{h}_{c}")
                        for jt in range(JT):
                            hh, j2 = divmod(jt, JT // 2)
                            nc.tensor.matmul(
                                av[0:DK + 1, :],
                                v_sb[hh][:, j2, h, :],
                                exp_tiles[jt // JG][:, jt % JG, :],
                                start=(jt == 0), stop=(jt == JT - 1),
                            )
                        st = stp.tile([128, 512], F32, tag="st", name=f"st{h}_{c}")
                        nc.vector.tensor_copy(st[0:DK + 1, :], av[0:DK + 1, :])
                        av_stages.append(st)

                    # denominators: gather rowsum rows, reciprocal, broadcast
                    rsg = small.tile([16, 128], F32, tag=f"rsg{h % 2}",
                                     name=f"rsg{h}")
                    for c in range(QC):
                        nc.sync.dma_start(
                            rsg[c * 4:(c + 1) * 4, :],
                            av_stages[c][DK:DK + 1, :].rearrange(
                                "one (p f) -> one p f", p=4),
                        )
                    nc.vector.reciprocal(rsg, rsg)
                    rrow = small.tile([1, S], F32R, tag=f"rrow{h % 2}",
                                      name=f"rrow{h}")
                    nc.sync.dma_start(
                        rrow.rearrange("one (p f) -> one p f", p=16),
                        rsg.bitcast(F32R),
                    )
                    for c in range(QC):
                        csl = slice(c * 512, (c + 1) * 512)
                        rps = ps_r.tile([64, 512], F32, tag="rps", name=f"rps{h}_{c}")
                        nc.tensor.matmul(
                            rps, ones_r[0:1, :], rrow[0:1, csl],
                            start=True, stop=True,
                        )
                        nc.vector.tensor_tensor(
                            outt_sb[p][hsl, csl],
                            av_stages[c][0:DK, :],
                            rps,
                            mybir.AluOpType.mult,
                        )

                # phase C: partial = outT^T @ Wo_shard
                for qt in range(S // 128):
                    osb = work.tile([128, 1024], F32, tag="osb", name=f"osb{qt}")
                    for ch in range(2):
                        acc = ps_wo.tile([128, 512], F32, tag="po",
                                         name=f"po{qt}_{ch}")
                        for p in range(2):
                            nc.tensor.matmul(
                                acc,
                                outt_sb[p][:, qt * 128:(qt + 1) * 128],
                                wo_sb[p][:, ch * 512:(ch + 1) * 512],
                                start=(p == 0), stop=(p == 1),
                            )
                        nc.vector.tensor_copy(osb[:, ch * 512:(ch + 1) * 512], acc)
                    nc.sync.dma_start(out[qt * 128:(qt + 1) * 128, :], osb)

    nc.compile()
    return nc


_NC = None


def _get_nc():
    global _NC
    if _NC is None:
        _NC = _build_nc()
    return _NC


def make_in_maps(query, key, value, Wq, Wk, Wv, Wo):
    query = _round_fp32r(np.ascontiguousarray(query, dtype=np.float32))
    key_ = _round_fp32r(np.ascontiguousarray(key, dtype=np.float32))
    value = _round_fp32r(np.ascontiguousarray(value, dtype=np.float32))
    Wq = _round_fp32r(np.ascontiguousarray(Wq, dtype=np.float32))
    Wk = _round_fp32r(np.ascontiguousarray(Wk, dtype=np.float32))
    Wv = _round_fp32r(np.ascontiguousarray(Wv, dtype=np.float32))
    Wo = _round_fp32r(np.ascontiguousarray(Wo, dtype=np.float32))

    xqT = [np.ascontiguousarray(query[b].T) for b in range(B)]
    xkT = [np.ascontiguousarray(key_[b].T) for b in range(B)]
    xvT = [np.ascontiguousarray(value[b].T) for b in range(B)]

    in_maps = []
    for core in range(8):
        b, g = divmod(core, 4)
        sl = slice(g * DG, (g + 1) * DG)
        in_maps.append({
            "xq": xqT[b],
            "xk": xkT[b],
            "xv": xvT[b],
            "wq": np.ascontiguousarray(Wq[:, sl]),
            "wk": np.ascontiguousarray(Wk[:, sl]),
            "wv": np.ascontiguousarray(Wv[:, sl]),
            "wo": np.ascontiguousarray(Wo[sl, :]),
        })
    return in_maps


def combine_results(results):
    out = np.zeros((B, S, D), dtype=np.float32)
    for core in range(8):
        out[core // 4] += results[core]["out"]
    return out


def kernel(query, key, value, Wq, Wk, Wv, Wo, _trace=False):
    from concourse import bass_utils

    nc = _get_nc()
    in_maps = make_in_maps(query, key, value, Wq, Wk, Wv, Wo)
    r = bass_utils.run_bass_kernel_spmd(
        nc, in_maps, core_ids=list(range(8)), trace=_trace
    )
    kernel.last_results = r
    return combine_results(r.results)


# revision 4
# speedup vs baseline: 1.3055x; 1.3055x over previous
"""Trainium2 Bass kernel for 16-head MultiHeadAttention (B=2, S=2048, D=1024, f32).

Sharding: 8 cores = 2 (batch) x 4 (head groups of 4 heads).
Each core gets a col-shard of Wq/Wk/Wv ([1024,256]) + row-shard of Wo ([256,1024]),
computes a full [2048,1024] partial output; the host sums 4 partials per batch.

On-device pipeline (transposed layouts, seq on the free axis):
  QT/KT = Wpair^T @ x^T            -> [128(=2 heads x 64), 2048] f32r
  VT    = Wv_pair^T @ xv^T, then PE-transposed to V_aug [j, head, 65]
          (65th column = ones so AV emits softmax denominators for free)
  sT    = KT_h^T-slice @ QT_h-slice, two heads row-packed in the PE array
          concurrently via tile_position (0,0)/(64,0)
  expT  = exp(0.125 * sT) via ACT from PSUM [128,2048] spans -> f32r
  outT/rowsum = V_aug^T @ expT          (M=65: rows 0-63 outT, row 64 rowsum)
  recip: rowsum rows -> DMA partition-gather -> lane-parallel reciprocal
         -> K=1 ones-matmul broadcast -> multiplied into outT (f32r)
  partial = outT^T @ Wo_shard           (K=128 over stacked head pairs)

All matmuls run as float32r (TF32-like 11-bit mantissa): full PE speed at
near-fp32 accuracy. Host pre-rounds inputs to the fp32r grid.
"""

import sys

import numpy as np

if "/opt/trn_rl_repo" not in sys.path:
    sys.path.insert(0, "/opt/trn_rl_repo")

import concourse.bacc as bacc
import concourse.mybir as mybir
import concourse.tile as tile
from concourse.masks import make_identity

F32 = mybir.dt.float32
F32R = mybir.dt.float32r

B, S, D, H = 2, 2048, 1024, 16
DK = D // H          # 64
HL = 4               # heads per core
DG = HL * DK         # 256
SCALE = 0.125        # 1/sqrt(DK)

ET = D // 128        # 8 e-tiles
JT = S // 128        # 16 j-tiles
QC = S // 512        # 4 q-chunks


def _round_fp32r(x: np.ndarray) -> np.ndarray:
    """Round f32 to the fp32r grid (11-bit mantissa, RNE), like walrus fp32_to_fp32r."""
    u = x.view(np.uint32).astype(np.uint64)
    u = (u + 0x7FF + ((u >> 12) & 1)) & np.uint64(0xFFFFF000)
    return u.astype(np.uint32).view(np.float32)


def _build_nc():
    nc = bacc.Bacc("TRN2", target_bir_lowering=False, debug=False)

    xq = nc.dram_tensor("xq", [D, S], F32, kind="ExternalInput").ap()
    xk = nc.dram_tensor("xk", [D, S], F32, kind="ExternalInput").ap()
    xv = nc.dram_tensor("xv", [D, S], F32, kind="ExternalInput").ap()
    wq = nc.dram_tensor("wq", [D, DG], F32, kind="ExternalInput").ap()
    wk = nc.dram_tensor("wk", [D, DG], F32, kind="ExternalInput").ap()
    wv = nc.dram_tensor("wv", [D, DG], F32, kind="ExternalInput").ap()
    wo = nc.dram_tensor("wo", [DG, D], F32, kind="ExternalInput").ap()
    out = nc.dram_tensor("out", [S, D], F32, kind="ExternalOutput").ap()

    with tile.TileContext(nc) as tc:
        with (
            tc.tile_pool(name="wpool", bufs=1) as wpool,
            tc.tile_pool(name="xin", bufs=3) as xin,
            tc.tile_pool(name="proj", bufs=1) as proj,
            tc.tile_pool(name="expp", bufs=3) as expp,
            tc.tile_pool(name="stp", bufs=10) as stp,
            tc.tile_pool(name="work", bufs=2) as work,
            tc.tile_pool(name="small", bufs=1) as small,
        ):
            # ---- constants / resident weights ------------------------------
            wq_sb = [wpool.tile([128, DG], F32R, tag=f"wq{e}", name=f"wq{e}")
                     for e in range(ET)]
            wk_sb = [wpool.tile([128, DG], F32R, tag=f"wk{e}", name=f"wk{e}")
                     for e in range(ET)]
            wv_sb = [wpool.tile([128, DG], F32R, tag=f"wv{e}", name=f"wv{e}")
                     for e in range(ET)]
            for e in range(ET):
                sl = slice(e * 128, (e + 1) * 128)
                nc.gpsimd.dma_start(wq_sb[e], wq.bitcast(F32R)[sl, :])
                nc.gpsimd.dma_start(wk_sb[e], wk.bitcast(F32R)[sl, :])
                nc.gpsimd.dma_start(wv_sb[e], wv.bitcast(F32R)[sl, :])
            wo_sb = [wpool.tile([128, D], F32R, tag=f"wo{p}", name=f"wo{p}")
                     for p in range(2)]
            for p in range(2):
                nc.gpsimd.dma_start(wo_sb[p], wo.bitcast(F32R)[p * 128:(p + 1) * 128, :])

            ones16 = wpool.tile([16, 64], F32, tag="ones16", name="ones16")
            nc.vector.memset(ones16, 1.0)
            ones_r = wpool.tile([16, 64], F32R, tag="ones_r", name="ones_r")
            nc.vector.tensor_copy(ones_r, ones16)
            ones_col = wpool.tile([128, 64], F32, tag="ones_col", name="ones_col")
            nc.vector.memset(ones_col, 1.0)
            ident_f = wpool.tile([128, 128], F32, tag="ident_f", name="ident_f")
            make_identity(nc, ident_f)
            ident = wpool.tile([128, 128], F32R, tag="ident", name="ident")
            nc.vector.tensor_copy(ident, ident_f)

            # ---- persistent activation tiles -------------------------------
            kt_sb = [proj.tile([128, S], F32R, tag=f"kt{p}", name=f"kt{p}")
                     for p in range(2)]
            qt_sb = [proj.tile([128, S], F32R, tag=f"qt{p}", name=f"qt{p}")
                     for p in range(2)]
            # V_aug: [j=128, j2, head, 65], two tiles cover jt = 0..7 / 8..15
            v_sb = [proj.tile([128, JT // 2, HL, DK + 1], F32R,
                              tag=f"v{hh}", name=f"v{hh}") for hh in range(2)]

            # ---- phase A: projections (8 PSUM accumulators, x streamed) ----
            with tc.tile_pool(name="ps_a", bufs=1, space="PSUM") as ps_a:
                def proj_pairs(x_dram, w_tiles, dst_tiles, nm):
                    accs = [ps_a.tile([128, 512], F32, tag="pa", bufs=8,
                                      name=f"acc_{nm}_{p}_{c}")
                            for p in range(2) for c in range(QC)]
                    for e in range(ET):
                        xt = xin.tile([128, S], F32R, tag="xs", name=f"x_{nm}{e}")
                        eng = nc.sync if e % 2 == 0 else nc.scalar
                        eng.dma_start(
                            xt, x_dram.bitcast(F32R)[e * 128:(e + 1) * 128, :])
                        for p in range(2):
                            for c in range(QC):
                                nc.tensor.matmul(
                                    accs[p * QC + c],
                                    w_tiles[e][:, p * 128:(p + 1) * 128],
                                    xt[:, c * 512:(c + 1) * 512],
                                    start=(e == 0), stop=(e == ET - 1),
                                )
                    for p in range(2):
                        for c in range(QC):
                            nc.vector.tensor_copy(
                                dst_tiles[p][:, c * 512:(c + 1) * 512],
                                accs[p * QC + c],
                            )

                proj_pairs(xk, wk_sb, kt_sb, "k")
                proj_pairs(xq, wq_sb, qt_sb, "q")
                vt_sb = [proj.tile([128, S], F32R, tag=f"ot{p}", name=f"vt{p}")
                         for p in range(2)]
                proj_pairs(xv, wv_sb, vt_sb, "v")

            with tc.tile_pool(name="ps_t", bufs=2, space="PSUM") as ps_t:
                for p in range(2):
                    for jt in range(JT):
                        pt = ps_t.tile([128, 128], F32R, tag="pt", name=f"pt{p}_{jt}")
                        nc.tensor.transpose(
                            pt, vt_sb[p][:, jt * 128:(jt + 1) * 128], ident)
                        hh, j2 = divmod(jt, JT // 2)
                        nc.vector.tensor_copy(
                            v_sb[hh][:, j2, 2 * p:2 * p + 2, 0:DK],
                            pt.rearrange("j (h d) -> j h d", h=2),
                        )
                for hh in range(2):
                    nc.vector.tensor_copy(
                        v_sb[hh][:, :, :, DK:DK + 1],
                        ones_col[:, 0:32].rearrange("p (a b) -> p a b", a=8)[:, :, :, None],
                    )

            # ---- phases B+C: attention + output projection ------------------
            outt_sb = [proj.tile([128, S], F32R, tag=f"ot{p}", name=f"outt{p}")
                       for p in range(2)]

            with (
                tc.tile_pool(name="ps_sc", bufs=1, space="PSUM") as ps_sc,
                tc.tile_pool(name="ps_av", bufs=2, space="PSUM") as ps_av,
                tc.tile_pool(name="ps_r", bufs=1, space="PSUM") as ps_r,
                tc.tile_pool(name="ps_wo", bufs=1, space="PSUM") as ps_wo,
            ):
                for p in range(2):
                    hA, hB = 2 * p, 2 * p + 1
                    stages = {}          # (c, i) -> staged [65, 512] tile
                    for c in range(QC):
                        csl = slice(c * 512, (c + 1) * 512)
                        # scores: 4 groups of (2 jt x 2 heads), row-packed pairs
                        exp_tiles = []
                        for g in range(JT // 2):
                            sc = ps_sc.tile([128, 4 * 512], F32, tag="sc",
                                            name=f"sc{p}_{c}_{g}")
                            for jj in range(2):
                                jt = g * 2 + jj
                                jsl = slice(jt * 128, (jt + 1) * 128)
                                nc.tensor.matmul(
                                    sc[:, (2 * jj) * 512:(2 * jj + 1) * 512],
                                    kt_sb[p][0:64, jsl],
                                    qt_sb[p][0:64, csl],
                                    start=True, stop=True,
                                    tile_position=(0, 0),
                                )
                                nc.tensor.matmul(
                                    sc[:, (2 * jj + 1) * 512:(2 * jj + 2) * 512],
                                    kt_sb[p][64:128, jsl],
                                    qt_sb[p][64:128, csl],
                                    start=True, stop=True,
                                    tile_position=(64, 0),
                                )
                            ex = expp.tile([128, 4, 512], F32R, tag="ex",
                                           name=f"ex{p}_{c}_{g}")
                            nc.scalar.activation(
                                out=ex,
                                in_=sc.rearrange("j (t q) -> j t q", t=4),
                                func=mybir.ActivationFunctionType.Exp,
                                scale=SCALE,
                            )
                            exp_tiles.append(ex)
                        # AV for both heads (full-row K=128 accumulation)
                        avA = ps_av.tile([128, 512], F32, tag="av", name=f"avA{p}_{c}")
                        avB = ps_av.tile([128, 512], F32, tag="av", name=f"avB{p}_{c}")
                        for jt in range(JT):
                            hh, j2 = divmod(jt, JT // 2)
                            ex = exp_tiles[jt // 2]
                            slot = 2 * (jt % 2)
                            nc.tensor.matmul(
                                avA[0:DK + 1, :],
                                v_sb[hh][:, j2, hA, :],
                                ex[:, slot, :],
                                start=(jt == 0), stop=(jt == JT - 1),
                            )
                            nc.tensor.matmul(
                                avB[0:DK + 1, :],
                                v_sb[hh][:, j2, hB, :],
                                ex[:, slot + 1, :],
                                start=(jt == 0), stop=(jt == JT - 1),
                            )
                        for i, av in ((0, avA), (1, avB)):
                            st = stp.tile([128, 512], F32, tag="st",
                                          name=f"st{p}_{c}_{i}")
                            nc.vector.tensor_copy(st[0:DK + 1, :], av[0:DK + 1, :])
                            stages[(c, i)] = st

                    # denominators + normalize, per head of the pair
                    for i in range(2):
                        hsl = slice(i * 64, (i + 1) * 64)
                        rsg = small.tile([16, 128], F32, tag=f"rsg{i}",
                                         name=f"rsg{p}_{i}")
                        for c in range(QC):
                            nc.sync.dma_start(
                                rsg[c * 4:(c + 1) * 4, :],
                                stages[(c, i)][DK:DK + 1, :].rearrange(
                                    "one (pp f) -> one pp f", pp=4),
                            )
                        nc.vector.reciprocal(rsg, rsg)
                        rrow = small.tile([1, S], F32R, tag=f"rrow{i}",
                                          name=f"rrow{p}_{i}")
                        nc.sync.dma_start(
                            rrow.rearrange("one (pp f) -> one pp f", pp=16),
                            rsg.bitcast(F32R),
                        )
                        for c in range(QC):
                            csl = slice(c * 512, (c + 1) * 512)
                            rps = ps_r.tile([64, 512], F32, tag="rps",
                                            name=f"rps{p}_{i}_{c}")
                            nc.tensor.matmul(
                                rps, ones_r[0:1, :], rrow[0:1, csl],
                                start=True, stop=True,
                            )
                            nc.vector.tensor_tensor(
                                outt_sb[p][hsl, csl],
                                stages[(c, i)][0:DK, :],
                                rps,
                                mybir.AluOpType.mult,
                            )

                # phase C: partial = outT^T @ Wo_shard
                for qt in range(S // 128):
                    osb = work.tile([128, 1024], F32, tag="osb", name=f"osb{qt}")
                    for ch in range(2):
                        acc = ps_wo.tile([128, 512], F32, tag="po",
                                         name=f"po{qt}_{ch}")
                        for p in range(2):
                            nc.tensor.matmul(
                                acc,
                                outt_sb[p][:, qt * 128:(qt + 1) * 128],
                                wo_sb[p][:, ch * 512:(ch + 1) * 512],
                                start=(p == 0), stop=(p == 1),
                            )
                        nc.vector.tensor_copy(osb[:, ch * 512:(ch + 1) * 512], acc)
                    nc.sync.dma_start(out[qt * 128:(qt + 1) * 128, :], osb)

    nc.compile()
    return nc


_NC = None


def _get_nc():
    global _NC
    if _NC is None:
        _NC = _build_nc()
    return _NC


def make_in_maps(query, key, value, Wq, Wk, Wv, Wo):
    query = _round_fp32r(np.ascontiguousarray(query, dtype=np.float32))
    key_ = _round_fp32r(np.ascontiguousarray(key, dtype=np.float32))
    value = _round_fp32r(np.ascontiguousarray(value, dtype=np.float32))
    Wq = _round_fp32r(np.ascontiguousarray(Wq, dtype=np.float32))
    Wk = _round_fp32r(np.ascontiguousarray(Wk, dtype=np.float32))
    Wv = _round_fp32r(np.ascontiguousarray(Wv, dtype=np.float32))
    Wo = _round_fp32r(np.ascontiguousarray(Wo, dtype=np.float32))

    xqT = [np.ascontiguousarray(query[b].T) for b in range(B)]
    xkT = [np.ascontiguousarray(key_[b].T) for b in range(B)]
    xvT = [np.ascontiguousarray(value[b].T) for b in range(B)]

    in_maps = []
    for core in range(8):
        b, g = divmod(core, 4)
        sl = slice(g * DG, (g + 1) * DG)
        in_maps.append({
            "xq": xqT[b],
            "xk": xkT[b],
            "xv": xvT[b],
            "wq": np.ascontiguousarray(Wq[:, sl]),
            "wk": np.ascontiguousarray(Wk[:, sl]),
            "wv": np.ascontiguousarray(Wv[:, sl]),
            "wo": np.ascontiguousarray(Wo[sl, :]),
        })
    return in_maps


def combine_results(results):
    out = np.zeros((B, S, D), dtype=np.float32)
    for core in range(8):
        out[core // 4] += results[core]["out"]
    return out


def kernel(query, key, value, Wq, Wk, Wv, Wo, _trace=False):
    from concourse import bass_utils

    nc = _get_nc()
    in_maps = make_in_maps(query, key, value, Wq, Wk, Wv, Wo)
    r = bass_utils.run_bass_kernel_spmd(
        nc, in_maps, core_ids=list(range(8)), trace=_trace
    )
    kernel.last_results = r
    return combine_results(r.results)


# revision 5
# speedup vs baseline: 1.4743x; 1.1293x over previous
"""Trainium2 Bass kernel for 16-head MultiHeadAttention (B=2, S=2048, D=1024, f32).

Sharding: 8 cores = 2 (batch) x 4 (head groups of 4 heads).
Each core gets a col-shard of Wq/Wk/Wv ([1024,256]) + row-shard of Wo ([256,1024]),
computes a full [2048,1024] partial output; the host sums 4 partials per batch.

On-device pipeline (transposed layouts, seq on the free axis):
  QT/KT = Wpair^T @ x^T            -> [128(=2 heads x 64), 2048] f32r
  VT    = Wv_pair^T @ xv^T, then PE-transposed to V_aug [j, head, 65]
          (65th column = ones so AV emits softmax denominators for free)
  sT    = KT_h^T-slice @ QT_h-slice, two heads row-packed in the PE array
          concurrently via tile_position (0,0)/(64,0)
  expT  = exp(0.125 * sT) via ACT from PSUM [128,2048] spans -> f32r
  outT/rowsum = V_aug^T @ expT          (M=65: rows 0-63 outT, row 64 rowsum)
  recip: rowsum rows -> DMA partition-gather -> lane-parallel reciprocal
         -> K=1 ones-matmul broadcast -> multiplied into outT (f32r)
  partial = outT^T @ Wo_shard           (K=128 over stacked head pairs)

All matmuls run as float32r (TF32-like 11-bit mantissa): full PE speed at
near-fp32 accuracy. Host pre-rounds inputs to the fp32r grid.
"""

import sys

import numpy as np

if "/opt/trn_rl_repo" not in sys.path:
    sys.path.insert(0, "/opt/trn_rl_repo")

import concourse.bacc as bacc
import concourse.mybir as mybir
import concourse.tile as tile
from concourse.masks import make_identity

F32 = mybir.dt.float32
F32R = mybir.dt.float32r

B, S, D, H = 2, 2048, 1024, 16
DK = D // H          # 64
HL = 4               # heads per core
DG = HL * DK         # 256
SCALE = 0.125        # 1/sqrt(DK)

ET = D // 128        # 8 e-tiles
JT = S // 128        # 16 j-tiles
QC = S // 512        # 4 q-chunks


def _round_fp32r(x: np.ndarray) -> np.ndarray:
    """Round f32 to the fp32r grid (11-bit mantissa, RNE), like walrus fp32_to_fp32r."""
    u = x.view(np.uint32).astype(np.uint64)
    u = (u + 0x7FF + ((u >> 12) & 1)) & np.uint64(0xFFFFF000)
    return u.astype(np.uint32).view(np.float32)


def _build_nc():
    nc = bacc.Bacc("TRN2", target_bir_lowering=False, debug=False)

    xq = nc.dram_tensor("xq", [D, S], F32, kind="ExternalInput").ap()
    xk = nc.dram_tensor("xk", [D, S], F32, kind="ExternalInput").ap()
    xv = nc.dram_tensor("xv", [D, S], F32, kind="ExternalInput").ap()
    wq = nc.dram_tensor("wq", [D, DG], F32, kind="ExternalInput").ap()
    wk = nc.dram_tensor("wk", [D, DG], F32, kind="ExternalInput").ap()
    wv = nc.dram_tensor("wv", [D, DG], F32, kind="ExternalInput").ap()
    wo = nc.dram_tensor("wo", [DG, D], F32, kind="ExternalInput").ap()
    out = nc.dram_tensor("out", [S, D], F32, kind="ExternalOutput").ap()

    with tile.TileContext(nc) as tc:
        with (
            tc.tile_pool(name="wpool", bufs=1) as wpool,
            tc.tile_pool(name="xin", bufs=3) as xin,
            tc.tile_pool(name="proj", bufs=1) as proj,
            tc.tile_pool(name="expp", bufs=3) as expp,
            tc.tile_pool(name="stp", bufs=10) as stp,
            tc.tile_pool(name="work", bufs=2) as work,
            tc.tile_pool(name="small", bufs=1) as small,
        ):
            # ---- constants / resident weights ------------------------------
            wq_sb = [wpool.tile([128, DG], F32R, tag=f"wq{e}", name=f"wq{e}")
                     for e in range(ET)]
            wk_sb = [wpool.tile([128, DG], F32R, tag=f"wk{e}", name=f"wk{e}")
                     for e in range(ET)]
            wv_sb = [wpool.tile([128, DG], F32R, tag=f"wv{e}", name=f"wv{e}")
                     for e in range(ET)]
            for e in range(ET):
                sl = slice(e * 128, (e + 1) * 128)
                nc.gpsimd.dma_start(wq_sb[e], wq.bitcast(F32R)[sl, :])
                nc.gpsimd.dma_start(wk_sb[e], wk.bitcast(F32R)[sl, :])
                nc.gpsimd.dma_start(wv_sb[e], wv.bitcast(F32R)[sl, :])
            wo_sb = [wpool.tile([128, D], F32R, tag=f"wo{p}", name=f"wo{p}")
                     for p in range(2)]
            for p in range(2):
                nc.gpsimd.dma_start(wo_sb[p], wo.bitcast(F32R)[p * 128:(p + 1) * 128, :])

            ones16 = wpool.tile([16, 64], F32, tag="ones16", name="ones16")
            nc.vector.memset(ones16, 1.0)
            ones_r = wpool.tile([16, 64], F32R, tag="ones_r", name="ones_r")
            nc.vector.tensor_copy(ones_r, ones16)
            ones_col = wpool.tile([128, 64], F32, tag="ones_col", name="ones_col")
            nc.vector.memset(ones_col, 1.0)
            ident_f = wpool.tile([128, 128], F32, tag="ident_f", name="ident_f")
            make_identity(nc, ident_f)
            ident = wpool.tile([128, 128], F32R, tag="ident", name="ident")
            nc.vector.tensor_copy(ident, ident_f)

            # ---- persistent activation tiles -------------------------------
            kt_sb = [proj.tile([128, S], F32R, tag=f"kt{p}", name=f"kt{p}")
                     for p in range(2)]
            qt_sb = [proj.tile([128, S], F32R, tag=f"qt{p}", name=f"qt{p}")
                     for p in range(2)]
            # V_aug: [j=128, j2, head, 65], two tiles cover jt = 0..7 / 8..15
            v_sb = [proj.tile([128, JT // 2, HL, DK + 1], F32R,
                              tag=f"v{hh}", name=f"v{hh}") for hh in range(2)]

            # ---- phase A: projections (8 PSUM accumulators, x streamed) ----
            with tc.tile_pool(name="ps_a", bufs=1, space="PSUM") as ps_a:
                def proj_pairs(x_dram, w_tiles, dst_tiles, nm):
                    accs = [ps_a.tile([128, 512], F32, tag="pa", bufs=8,
                                      name=f"acc_{nm}_{p}_{c}")
                            for p in range(2) for c in range(QC)]
                    for e in range(ET):
                        xt = xin.tile([128, S], F32R, tag="xs", name=f"x_{nm}{e}")
                        eng = nc.sync if e % 2 == 0 else nc.scalar
                        eng.dma_start(
                            xt, x_dram.bitcast(F32R)[e * 128:(e + 1) * 128, :])
                        for p in range(2):
                            for c in range(QC):
                                nc.tensor.matmul(
                                    accs[p * QC + c],
                                    w_tiles[e][:, p * 128:(p + 1) * 128],
                                    xt[:, c * 512:(c + 1) * 512],
                                    start=(e == 0), stop=(e == ET - 1),
                                )
                    for p in range(2):
                        for c in range(QC):
                            nc.vector.tensor_copy(
                                dst_tiles[p][:, c * 512:(c + 1) * 512],
                                accs[p * QC + c],
                            )

                proj_pairs(xk, wk_sb, kt_sb, "k")
                proj_pairs(xq, wq_sb, qt_sb, "q")
                vt_sb = [proj.tile([128, S], F32R, tag=f"ot{p}", name=f"vt{p}")
                         for p in range(2)]
                proj_pairs(xv, wv_sb, vt_sb, "v")

            with tc.tile_pool(name="ps_t", bufs=2, space="PSUM") as ps_t:
                for p in range(2):
                    for jt in range(JT):
                        pt = ps_t.tile([128, 128], F32R, tag="pt", name=f"pt{p}_{jt}")
                        nc.tensor.transpose(
                            pt, vt_sb[p][:, jt * 128:(jt + 1) * 128], ident)
                        hh, j2 = divmod(jt, JT // 2)
                        nc.vector.tensor_copy(
                            v_sb[hh][:, j2, 2 * p:2 * p + 2, 0:DK],
                            pt.rearrange("j (h d) -> j h d", h=2),
                        )
                for hh in range(2):
                    nc.vector.tensor_copy(
                        v_sb[hh][:, :, :, DK:DK + 1],
                        ones_col[:, 0:32].rearrange("p (a b) -> p a b", a=8)[:, :, :, None],
                    )

            # ---- phases B+C: attention + output projection ------------------
            outt_sb = [proj.tile([128, S], F32R, tag=f"ot{p}", name=f"outt{p}")
                       for p in range(2)]

            with (
                tc.tile_pool(name="ps_sc", bufs=2, space="PSUM") as ps_sc,
                tc.tile_pool(name="ps_av", bufs=3, space="PSUM") as ps_av,
                tc.tile_pool(name="ps_wo", bufs=1, space="PSUM") as ps_wo,
            ):
                for p in range(2):
                    hA, hB = 2 * p, 2 * p + 1
                    stages = {}          # (c, i) -> staged [65, 512] tile
                    for c in range(QC):
                        csl = slice(c * 512, (c + 1) * 512)
                        # scores: 16 groups of (1 jt x 2 heads), row-packed pairs
                        exp_tiles = []
                        for jt in range(JT):
                            jsl = slice(jt * 128, (jt + 1) * 128)
                            sc = ps_sc.tile([128, 2 * 512], F32, tag="sc",
                                            name=f"sc{p}_{c}_{jt}")
                            nc.tensor.matmul(
                                sc[:, 0:512],
                                kt_sb[p][0:64, jsl],
                                qt_sb[p][0:64, csl],
                                start=True, stop=True,
                                tile_position=(0, 0),
                            )
                            nc.tensor.matmul(
                                sc[:, 512:1024],
                                kt_sb[p][64:128, jsl],
                                qt_sb[p][64:128, csl],
                                start=True, stop=True,
                                tile_position=(64, 0),
                            )
                            ex = expp.tile([128, 2, 512], F32R, tag="ex",
                                           name=f"ex{p}_{c}_{jt}")
                            nc.scalar.activation(
                                out=ex,
                                in_=sc.rearrange("j (t q) -> j t q", t=2),
                                func=mybir.ActivationFunctionType.Exp,
                                scale=SCALE,
                            )
                            exp_tiles.append(ex)
                        # AV for both heads (full-row K=128 accumulation)
                        avA = ps_av.tile([128, 512], F32, tag="av", name=f"avA{p}_{c}")
                        avB = ps_av.tile([128, 512], F32, tag="av", name=f"avB{p}_{c}")
                        for jt in range(JT):
                            hh, j2 = divmod(jt, JT // 2)
                            ex = exp_tiles[jt]
                            nc.tensor.matmul(
                                avA[0:DK + 1, :],
                                v_sb[hh][:, j2, hA, :],
                                ex[:, 0, :],
                                start=(jt == 0), stop=(jt == JT - 1),
                            )
                            nc.tensor.matmul(
                                avB[0:DK + 1, :],
                                v_sb[hh][:, j2, hB, :],
                                ex[:, 1, :],
                                start=(jt == 0), stop=(jt == JT - 1),
                            )
                        for i, av in ((0, avA), (1, avB)):
                            st = stp.tile([128, 512], F32, tag="st",
                                          name=f"st{p}_{c}_{i}")
                            nc.vector.tensor_copy(st[0:DK + 1, :], av[0:DK + 1, :])
                            stages[(c, i)] = st

                    # denominators + normalize, per head of the pair
                    for i in range(2):
                        hsl = slice(i * 64, (i + 1) * 64)
                        rsg = small.tile([16, 128], F32, tag=f"rsg{i}",
                                         name=f"rsg{p}_{i}")
                        for c in range(QC):
                            nc.sync.dma_start(
                                rsg[c * 4:(c + 1) * 4, :],
                                stages[(c, i)][DK:DK + 1, :].rearrange(
                                    "one (pp f) -> one pp f", pp=4),
                            )
                        nc.vector.reciprocal(rsg, rsg)
                        rrow = small.tile([1, S], F32R, tag=f"rrow{i}",
                                          name=f"rrow{p}_{i}")
                        nc.sync.dma_start(
                            rrow.rearrange("one (pp f) -> one pp f", pp=16),
                            rsg.bitcast(F32R),
                        )
                        for c in range(QC):
                            csl = slice(c * 512, (c + 1) * 512)
                            rps = ps_av.tile([64, 512], F32, tag="av",
                                             name=f"rps{p}_{i}_{c}")
                            nc.tensor.matmul(
                                rps, ones_r[0:1, :], rrow[0:1, csl],
                                start=True, stop=True,
                            )
                            nc.vector.tensor_tensor(
                                outt_sb[p][hsl, csl],
                                stages[(c, i)][0:DK, :],
                                rps,
                                mybir.AluOpType.mult,
                            )

                    # phase C (split per pair): partial += outT_p^T @ Wo_p
                    # pair 0 writes, pair 1 accumulates via DMA accum -> the
                    # pair-0 Wo matmuls overlap pair-1 attention.
                    for qt in range(S // 128):
                        osb = work.tile([128, 1024], F32, tag="osb",
                                        name=f"osb{p}_{qt}")
                        for ch in range(2):
                            acc = ps_wo.tile([128, 512], F32, tag="po",
                                             name=f"po{p}_{qt}_{ch}")
                            nc.tensor.matmul(
                                acc,
                                outt_sb[p][:, qt * 128:(qt + 1) * 128],
                                wo_sb[p][:, ch * 512:(ch + 1) * 512],
                                start=True, stop=True,
                            )
                            nc.vector.tensor_copy(osb[:, ch * 512:(ch + 1) * 512], acc)
                        if p == 0:
                            nc.sync.dma_start(out[qt * 128:(qt + 1) * 128, :], osb)
                        else:
                            nc.gpsimd.dma_start(
                                out[qt * 128:(qt + 1) * 128, :], osb,
                                accum_op=mybir.AluOpType.add,
                            )

    nc.compile()
    return nc


_NC = None


def _get_nc():
    global _NC
    if _NC is None:
        _NC = _build_nc()
    return _NC


def make_in_maps(query, key, value, Wq, Wk, Wv, Wo):
    query = _round_fp32r(np.ascontiguousarray(query, dtype=np.float32))
    key_ = _round_fp32r(np.ascontiguousarray(key, dtype=np.float32))
    value = _round_fp32r(np.ascontiguousarray(value, dtype=np.float32))
    Wq = _round_fp32r(np.ascontiguousarray(Wq, dtype=np.float32))
    Wk = _round_fp32r(np.ascontiguousarray(Wk, dtype=np.float32))
    Wv = _round_fp32r(np.ascontiguousarray(Wv, dtype=np.float32))
    Wo = _round_fp32r(np.ascontiguousarray(Wo, dtype=np.float32))

    xqT = [np.ascontiguousarray(query[b].T) for b in range(B)]
    xkT = [np.ascontiguousarray(key_[b].T) for b in range(B)]
    xvT = [np.ascontiguousarray(value[b].T) for b in range(B)]

    in_maps = []
    for core in range(8):
        b, g = divmod(core, 4)
        sl = slice(g * DG, (g + 1) * DG)
        in_maps.append({
            "xq": xqT[b],
            "xk": xkT[b],
            "xv": xvT[b],
            "wq": np.ascontiguousarray(Wq[:, sl]),
            "wk": np.ascontiguousarray(Wk[:, sl]),
            "wv": np.ascontiguousarray(Wv[:, sl]),
            "wo": np.ascontiguousarray(Wo[sl, :]),
        })
    return in_maps


def combine_results(results):
    out = np.zeros((B, S, D), dtype=np.float32)
    for core in range(8):
        out[core // 4] += results[core]["out"]
    return out


def kernel(query, key, value, Wq, Wk, Wv, Wo, _trace=False):
    from concourse import bass_utils

    nc = _get_nc()
    in_maps = make_in_maps(query, key, value, Wq, Wk, Wv, Wo)
    r = bass_utils.run_bass_kernel_spmd(
        nc, in_maps, core_ids=list(range(8)), trace=_trace
    )
    kernel.last_results = r
    return combine_results(r.results)
